# revision 17
# baseline (speedup 1.0000x reference)
"""Multi-head attention (2D-RoPE, masked softmax) on 8 Trainium2 NeuronCores.

Sharding: 4 head-groups (3 heads each) x 2 query-halves (1160 rows each).
Each core computes full attention for its 3 heads over its 1160 query rows
against all 2320 keys, plus its share of the output projection; the host
sums the 8 partial projections and adds the (folded) biases.

Device-side notes:
  - Attention core runs in bf16 (x, Wqkv, K/Q/V, rope tables, exp weights);
    v2: the output projection runs in bf16 as well (ctxn + Wproj) -- the
    f32r path hit the slow fp32 LDWEIGHTS mode.
  - K/Q are produced directly in transposed [head_dim, seq] layout so
    scores come out as scoresT[m, l] with keys on partitions -> softmax
    needs no partition reduction: exp on ScalarE straight out of PSUM
    (0.125 scale folded in; no max-subtraction needed, |s*scale| < ~3),
    denominator via a ones-column appended to V (free PSUM row), and the
    mask folded into V (zeroed rows + mask-valued ones-column, the
    ones-column written once per head instead of per key-chunk).
  - All attention matmuls are zero-padded to K=128 contraction: half-array
    (K=64) matmuls do not register as "busy" for the PE clock monitor and
    the kernel gets stuck at 1.2 GHz; full-array shapes hold 2.4 GHz.
  - Inner loop is software-pipelined over pairs of key-chunks (one exp op
    covers two chunks across two PSUM banks; ctx matmuls trail by two
    pairs) so ScalarE exp overlaps PE work; the 1/Z chain is deferred into
    the next tile's instruction stream (Z row -> reciprocal_approx ->
    K=1 outer-product matmul broadcast -> one DVE multiply).
  - Key order (m) is contraction-internal, so each core's x/K-tables/mask
    are permuted to put its own query rows first; one xT buffer serves
    both the K/V projections (all columns) and the Q projection (first
    1160 columns), keeping the program identical across cores (SPMD).
    x is passed in piece-major layout so every DMA is one contiguous
    block (descriptor-bound otherwise).
  - V-bias and output bias never touch the device:
    out = softmax(..) @ (Vx + bv) @ Wp.T + bp = dev_out + (Wp @ bv + bp).
"""
import sys
if '/opt/trn_rl_repo' not in sys.path:
    sys.path.insert(0, '/opt/trn_rl_repo')
import numpy as np

SEQ, E, NH, D = 2320, 768, 12, 64
GRID, TASK = 48, 16
SQ = SEQ // 2           # query rows per core
HG = 3                  # heads per core
SCALE = D ** -0.5
EC = 6                  # embed chunks of 128
L_TILES = [(0, 512), (512, 392), (904, 256)]
N_TILES = [(0, 512), (512, 512), (1024, 512), (1536, 512), (2048, 272)]
MC = [(i * 128, min(128, SEQ - i * 128)) for i in range(19)]
PT = [(i * 128, min(128, SQ - i * 128)) for i in range(10)]

_prog = None


def _build(stages=3):
    import concourse.mybir as mybir
    import concourse.tile as tile
    from concourse import bacc

    F32, F32R = mybir.dt.float32, mybir.dt.float32r
    BF16 = mybir.dt.bfloat16
    AF = mybir.ActivationFunctionType

    nc = bacc.Bacc('TRN2', target_bir_lowering=False, debug=False, num_devices=8)
    dp = nc.declare_dram_parameter
    xt_d = dp("xt", [4, E, 580], BF16, isOutput=False)
    wq_d = dp("wq", [E, 192], BF16, isOutput=False)
    wk_d = dp("wk", [E, 192], BF16, isOutput=False)
    wv_d = dp("wv", [E, 192], BF16, isOutput=False)
    wp_d = dp("wp", [192, E], BF16, isOutput=False)
    bq_d = dp("bq", [128, 2], F32, isOutput=False)
    bk_d = dp("bk", [128, 2], F32, isOutput=False)
    cq_d = dp("cq", [128, SQ], BF16, isOutput=False)
    sq_d = dp("sq", [128, SQ], BF16, isOutput=False)
    ck_d = dp("ck", [128, SEQ], BF16, isOutput=False)
    sk_d = dp("sk", [128, SEQ], BF16, isOutput=False)
    mk_d = dp("mk", [128, 19], F32, isOutput=False)
    on_d = dp("ones64", [1, 64], F32R, isOutput=False)
    out_d = dp("pout", [SQ, E], F32, isOutput=True)

    with tile.TileContext(nc) as tc:
        with (
            tc.tile_pool(name="long", bufs=1) as lp,
            tc.tile_pool(name="zp", bufs=2) as zp,
        ):
            kt01 = lp.tile([128, SEQ], BF16, tag="kt01")
            qt01 = lp.tile([128, SQ], BF16, tag="qt01")
            kt2 = lp.tile([128, SEQ], BF16, tag="kt2")
            qt2 = lp.tile([128, SQ], BF16, tag="qt2")
            rk01 = lp.tile([128, SEQ], BF16, tag="rk01")
            rq01 = lp.tile([128, SQ], BF16, tag="rq01")
            rk2 = lp.tile([64, SEQ], BF16, tag="rk2")
            rq2 = lp.tile([64, SQ], BF16, tag="rq2")
            v_all = lp.tile([128, 19, HG, 65], BF16, tag="v_all")
            wp01 = lp.tile([128, E], BF16, tag="wp01")
            wp2 = lp.tile([64, E], BF16, tag="wp2")
            ctxn01 = lp.tile([128, SQ], BF16, tag="ctxn01")
            ctxn2 = lp.tile([64, SQ], BF16, tag="ctxn2")
            ones64 = lp.tile([1, 64], F32R, tag="ones64")
            mk_sb = lp.tile([128, 19], F32, tag="mk")
            cq_sb = lp.tile([128, SQ], BF16, tag="cq")
            sq_sb = lp.tile([128, SQ], BF16, tag="sq")
            ck_sb = lp.tile([128, SEQ], BF16, tag="ck")
            sk_sb = lp.tile([128, SEQ], BF16, tag="sk")

            warm = lp.tile([128, 128], BF16, tag="warm")
            nc.gpsimd.memset(warm[:], 0.0)

            # small/ordering-critical DMAs first (weights, biases, tables),
            # split into pieces so consumers unblock early
            nc.sync.dma_start(ones64[:], on_d[:])
            nc.sync.dma_start(mk_sb[:], mk_d[:])
            nc.scalar.dma_start(wp01[:], wp_d[0:128, :])
            nc.scalar.dma_start(wp2[:], wp_d[128:192, :])
            nc.gpsimd.memset(kt2[64:128, :], 0.0)
            nc.gpsimd.memset(qt2[64:128, :], 0.0)

            # ---- phase A + attention: pools are scoped so lt0's attention
            # can emit before the V-tiles (which ride inside it), letting the
            # in-order PE queue reach the first scores ~30us earlier. ----
            with tc.tile_pool(name="p12", bufs=1) as p12:
                wq_sb = p12.tile([128, EC, 192], BF16, tag="wq")
                wk_sb = p12.tile([128, EC, 192], BF16, tag="wk")
                wv_sb = p12.tile([128, EC, 192], BF16, tag="wv")
                for w_sb, w_d in ((wk_sb, wk_d), (wq_sb, wq_d), (wv_sb, wv_d)):
                    nc.scalar.dma_start(
                        w_sb[:, :, :],
                        w_d[:, :].rearrange("(c p) w -> p c w", p=128))
                bq_sb = p12.tile([128, 2], F32, tag="bq")
                nc.sync.dma_start(bq_sb[:], bq_d[:])
                bk_sb = p12.tile([128, 2], F32, tag="bk")
                nc.sync.dma_start(bk_sb[:], bk_d[:])
                xt_all = p12.tile([128, EC, SEQ], BF16, tag="xta")
                for pi in range(4):
                    off = pi * 580
                    eng = nc.sync if pi < 2 else nc.scalar
                    eng.dma_start(
                        xt_all[:, :, off:off + 580],
                        xt_d[pi].rearrange("(c p) w -> p c w", p=128))
                nc.gpsimd.dma_start(ck_sb[:], ck_d[:])
                nc.gpsimd.dma_start(sk_sb[:], sk_d[:])
                nc.gpsimd.dma_start(cq_sb[:], cq_d[:])
                nc.gpsimd.dma_start(sq_sb[:], sq_d[:])
                nc.gpsimd.dma_start(wp01[:], wp_d[0:128, :])
                nc.gpsimd.dma_start(wp2[:], wp_d[128:192, :])

                # mask ones-column of v_all, written once per head
                for h in range(HG):
                    nc.vector.tensor_copy(
                        v_all[0:128, 0:19, h, 64:65],
                        mk_sb[0:128, 0:19].rearrange("p (a b) -> p a b", b=1))

                def v_tile(i):
                    off, m = MC[i]
                    pv = pvp.tile([128, 192], F32, tag="pv", name="pv")
                    for c in range(EC):
                        nc.tensor.matmul(
                            pv[0:m, :], xt_all[:, c, off:off + m], wv_sb[:, c, :],
                            start=(c == 0), stop=(c == EC - 1))
                    nc.vector.tensor_mul(
                        v_all[0:m, i, :, 0:64],
                        pv[0:m, 0:192].rearrange("p (h d) -> p h d", h=HG),
                        mk_sb[0:m, i:i + 1].to_broadcast([m, HG, 64]))

                def qk_mm(w_sb, b_sb, grp, off, n, rawt):
                    M = 128 if grp == 0 else 64
                    wc = slice(0, 128) if grp == 0 else slice(128, 192)
                    ps = pkp.tile([128, 512], F32, tag="pk", name="pk")
                    for c in range(EC):
                        nc.tensor.matmul(
                            ps[0:M, 0:n], w_sb[:, c, wc],
                            xt_all[:, c, off:off + n],
                            start=(c == 0), stop=(c == EC - 1))
                    bcol = 0 if grp == 0 else 1
                    nc.scalar.activation(
                        rawt[0:M, off:off + n], ps[0:M, 0:n],
                        AF.Identity,
                        bias=b_sb[0:M, bcol:bcol + 1],
                        scale=1.0)

                def dummy_mm(k=1):
                    for _ in range(k):
                        pwm = pkp.tile([128, 512], F32, tag="pk", name="pwm")
                        nc.tensor.matmul(pwm[0:128, 0:512],
                                         warm[0:128, 0:128],
                                         warm[0:128, 0:512],
                                         start=True, stop=True)

                # ---- attention pools (6 banks) span the K/Q phase and all
                # L-tiles; pkp (2 banks) closes after the K/Q jobs, pvp (2)
                # lives only for lt0's interleaved V-tiles, pp3 (2) after.
                with tc.tile_pool(name="ep", bufs=3) as ep, \
                     tc.tile_pool(name="op", bufs=2) as op, \
                     tc.tile_pool(name="rzp", bufs=4) as rzp, \
                     tc.tile_pool(name="ps3", bufs=2, space="PSUM") as ps3, \
                     tc.tile_pool(name="pc3", bufs=2, space="PSUM") as pc3:
                    with tc.tile_pool(name="pk", bufs=2, space="PSUM") as pkp:
                        # PE warmup on the pkp ring bridges the input-DMA
                        # window at full clock.
                        dummy_mm(16)
                        def piece_of(off, n):
                            return (off + n - 1) // 580
                        mixed = []
                        for pc in range(4):
                            for (off, n) in N_TILES:
                                if piece_of(off, n) == pc:
                                    mixed.append(("k", 0, off, n))
                                    mixed.append(("k", 1, off, n))
                            for (off, n) in L_TILES:
                                if piece_of(off, n) == pc:
                                    mixed.append(("q", 0, off, n))
                                    mixed.append(("q", 1, off, n))
                        for job in mixed:
                            kind, grp, off, n = job
                            if kind == "k":
                                qk_mm(wk_sb, bk_sb, grp, off, n,
                                      rk01 if grp == 0 else rk2)
                                if grp == 1:
                                    rope(rk01, rk01r, ck_sb, sk_sb, 128, off, n,
                                         [kt0, kt1])
                                    rope(rk2, rk2r, ck_sb, sk_sb, 64, off, n,
                                         [kt2])
                            else:
                                qk_mm(wq_sb, bq_sb, grp, off, n,
                                      rq01 if grp == 0 else rq2)
                                if grp == 1:
                                    rope(rq01, rq01r, cq_sb, sq_sb, 128, off, n,
                                         [qt0, qt1])
                                    rope(rq2, rq2r, cq_sb, sq_sb, 64, off, n,
                                         [qt2])

                    PROJ_OF_LT = {0: PT[0:4], 1: PT[4:7], 2: PT[7:10]}
                    PAIRS = [(i, i + 1) if i + 1 < len(MC) else (i,)
                             for i in range(0, len(MC), 2)]
                    pending = []
                    pp3 = None

                    def proj_slice(toff, tm):
                        outsb = op.tile([128, E], F32, tag="outsb", name="outsb")
                        for half in range(2):
                            hs = half * 384
                            pp = pp3.tile([128, 512], F32, tag="pp", name="pp")
                            nc.tensor.matmul(
                                pp[0:tm, 0:384], ctxn01[0:128, toff:toff + tm],
                                wp01[0:128, hs:hs + 384], start=True, stop=False)
                            nc.tensor.matmul(
                                pp[0:tm, 0:384], ctxn2[0:64, toff:toff + tm],
                                wp2[0:64, hs:hs + 384], start=False, stop=True)
                            nc.vector.tensor_copy(outsb[0:tm, hs:hs + 384],
                                                  pp[0:tm, 0:384])
                        nc.sync.dma_start(out_d[toff:toff + tm, :], outsb[0:tm, :])

                    def finish_tile(z):
                        zrow, ctxu, ctxap2, dr0, loff2, ln2 = z
                        zscr = zp.tile([1, 512], F32, tag="zscr", name="zscr")
                        rzf = zp.tile([1, 512], F32, tag="rzf", name="rzf")
                        nc.vector.reciprocal_approx_accurate(
                            rzf[0:1, 0:ln2], zrow[0:1, 0:ln2], zscr[0:1, 0:ln2])
                        rzr = zp.tile([1, 512], F32R, tag="rzr", name="rzr")
                        nc.vector.tensor_copy(rzr[0:1, 0:ln2], rzf[0:1, 0:ln2])
                        przb = pp3.tile([128, 512], F32, tag="pp", name="przb")
                        nc.tensor.matmul(
                            przb[0:64, 0:ln2], ones64[:], rzr[0:1, 0:ln2],
                            start=True, stop=True)
                        rzb = rzp.tile([64, 512], F32, tag="rzb", name="rzb")
                        nc.vector.tensor_copy(rzb[:, 0:ln2], przb[0:64, 0:ln2])
                        nc.vector.tensor_mul(
                            ctxap2[dr0:dr0 + 64, loff2:loff2 + ln2],
                            ctxu[0:64, 0:ln2], rzb[0:64, 0:ln2])

                    def park(pctx, dstt, dr0, loff, ln):
                        zrow = zp.tile([1, 512], F32, tag="zrow", name="zrow",
                                       bufs=4)
                        nc.vector.tensor_copy(zrow[0:1, 0:ln], pctx[64:65, 0:ln])
                        ctxu = rzp.tile([64, 512], F32, tag="ctxu", name="ctxu")
                        nc.vector.tensor_copy(ctxu[:, 0:ln], pctx[0:64, 0:ln])
                        pending.append((zrow, ctxu, dstt, dr0, loff, ln))

                    def exp_tile(ps, ex, m0, ln, two):
                        if two:
                            nc.scalar.activation(
                                ex[0:m0, 0:2 * ln].rearrange(
                                    "p (two n) -> p two n", two=2),
                                ps[0:m0, :].rearrange(
                                    "p (two n) -> p two n", two=2)[:, :, 0:ln],
                                AF.Exp, bias=0.0, scale=SCALE)
                        else:
                            nc.scalar.activation(
                                ex[0:m0, 0:ln], ps[0:m0, 0:ln], AF.Exp,
                                bias=0.0, scale=SCALE)

                    def lt_body(lt_i, loff, ln, emit_v):
                        # --- heads 0+1 ---
                        pctx01 = [pc3.tile([65, 512], F32, tag="pctx",
                                           name=f"pctx{h}") for h in range(2)]
                        exs = {}

                        def scores_exp01(p):
                            chunks = PAIRS[p]
                            for h, (ktap, qtap) in enumerate(
                                    ((kt0, qt0), (kt1, qt1))):
                                ps = ps3.tile([128, 1024], F32, tag="ps",
                                              name=f"ps{h}")
                                for j, i in enumerate(chunks):
                                    moff, m = MC[i]
                                    kk = 128 if m == 128 else 64
                                    nc.tensor.matmul(
                                        ps[0:m, j * 512:j * 512 + ln],
                                        ktap[0:kk, moff:moff + m],
                                        qtap[0:kk, loff:loff + ln],
                                        start=True, stop=True)
                                ex = ep.tile([128, 1024], BF16, tag="ex",
                                             name=f"ex{h}")
                                exp_tile(ps, ex, MC[chunks[0]][1], ln,
                                         len(chunks) == 2)
                                exs[(p, h)] = ex

                        def ctx01(p):
                            for h in range(2):
                                ex = exs.pop((p, h))
                                for j, i in enumerate(PAIRS[p]):
                                    moff, m = MC[i]
                                    nc.tensor.matmul(
                                        pctx01[h][:, 0:ln], v_all[0:m, i, h, :],
                                        ex[0:m, j * ln:j * ln + ln],
                                        start=(i == 0), stop=(i == len(MC) - 1))

                        if emit_v:
                            for i in range(4):
                                v_tile(i)
                        for p in range(len(PAIRS) + 2):
                            if emit_v:
                                for i in (2 * p + 4, 2 * p + 5):
                                    if i < len(MC):
                                        v_tile(i)
                            if p < len(PAIRS):
                                scores_exp01(p)
                            if p >= 2:
                                ctx01(p - 2)
                        park(pctx01[0], ctxn01, 0, loff, ln)
                        park(pctx01[1], ctxn01, 64, loff, ln)

                        # --- head 2: zero-padded K=128 path ---
                        pctx2 = pc3.tile([65, 512], F32, tag="pctx",
                                         name="pctx2")
                        exs2 = {}

                        def scores_exp2(p):
                            chunks = PAIRS[p]
                            ps = ps3.tile([128, 1024], F32, tag="ps", name="ps2")
                            for j, i in enumerate(chunks):
                                moff, m = MC[i]
                                kk = 128 if m == 128 else 64
                                nc.tensor.matmul(
                                    ps[0:m, j * 512:j * 512 + ln],
                                    kt2[0:kk, moff:moff + m],
                                    qt2[0:kk, loff:loff + ln],
                                    start=True, stop=True)
                            ex = ep.tile([128, 1024], BF16, tag="ex", name="ex2")
                            exp_tile(ps, ex, MC[chunks[0]][1], ln,
                                     len(chunks) == 2)
                            exs2[p] = ex

                        def ctx2(p):
                            ex = exs2.pop(p)
                            for j, i in enumerate(PAIRS[p]):
                                moff, m = MC[i]
                                nc.tensor.matmul(
                                    pctx2[:, 0:ln], v_all[0:m, i, 2, :],
                                    ex[0:m, j * ln:j * ln + ln],
                                    start=(i == 0), stop=(i == len(MC) - 1))

                        for p in range(len(PAIRS) + 2):
                            if p < len(PAIRS):
                                scores_exp2(p)
                            if p >= 2:
                                ctx2(p - 2)
                        park(pctx2, ctxn2, 0, loff, ln)

                    def flush_and_proj(lt_i):
                        while pending:
                            finish_tile(pending.pop(0))
                        for (toff, tm) in PROJ_OF_LT[lt_i]:
                            proj_slice(toff, tm)

                    with tc.tile_pool(name="pv", bufs=2, space="PSUM") as pvp:
                        lt_body(0, L_TILES[0][0], L_TILES[0][1], True)
                    with tc.tile_pool(name="pp3", bufs=2, space="PSUM") as pp3:
                        flush_and_proj(0)
                        for lt_i in (1, 2):
                            lt_body(lt_i, L_TILES[lt_i][0], L_TILES[lt_i][1],
                                    False)
                            flush_and_proj(lt_i)

    nc.finalize()
    return nc


def _rope_tables():
    dim = D // 2
    freqs = 1.0 / 10000 ** (np.arange(0, dim, 2, dtype=np.float64) / dim)
    t = np.arange(GRID, dtype=np.float64)
    f = np.repeat(np.outer(t, freqs), 2, axis=-1)                  # [48, 32]
    fr = np.broadcast_to(f[:, None, :], (GRID, GRID, dim))
    fc = np.broadcast_to(f[None, :, :], (GRID, GRID, dim))
    full = np.concatenate([fr, fc], axis=-1).reshape(GRID * GRID, D)
    cos = np.ones((SEQ, D), np.float64)
    sin = np.zeros((SEQ, D), np.float64)
    cos[TASK:] = np.cos(full)
    sin[TASK:] = np.sin(full)
    return cos.astype(np.float32), sin.astype(np.float32)


def _signed_stack(tT):
    # [64, S] -> [128, S]: signed sine table stored at the ROTATED (source)
    # rows, so the rope half-multiplies read both operands at equal partition
    # bases: sinB[32:64] = -sin[0:32], sinB[0:32] = +sin[32:64], stacked x2.
    s = np.vstack([tT[32:64], -tT[0:32]])
    return np.ascontiguousarray(np.vstack([s, s]))


def _core_inputs(x, mask, Wqkv, Wproj, bqkv, cos, sin, g, s):
    xT = x.T  # [768, 2320]
    q0 = SQ * s
    if s == 0:
        perm = None
        xt = np.ascontiguousarray(xT)
    else:
        perm = np.concatenate([np.arange(SQ, SEQ), np.arange(0, SQ)])
        xt = np.ascontiguousarray(np.concatenate([xT[:, SQ:], xT[:, :SQ]], axis=1))
    r0 = 192 * g
    wq = np.ascontiguousarray(Wqkv[r0:r0 + 192, :].T)
    wk = np.ascontiguousarray(Wqkv[768 + r0:768 + r0 + 192, :].T)
    wv = np.ascontiguousarray(Wqkv[1536 + r0:1536 + r0 + 192, :].T)
    wp = np.ascontiguousarray(Wproj[:, r0:r0 + 192].T)
    bq = np.zeros((128, 2), np.float32)
    bq[:, 0] = bqkv[r0:r0 + 128]
    bq[0:64, 1] = bqkv[r0 + 128:r0 + 192]
    bk = np.zeros((128, 2), np.float32)
    bk[:, 0] = bqkv[768 + r0:768 + r0 + 128]
    bk[0:64, 1] = bqkv[768 + r0 + 128:768 + r0 + 192]
    cosT, sinT = cos.T, sin.T  # [64, S]
    cq = np.ascontiguousarray(np.vstack([cosT, cosT])[:, q0:q0 + SQ])
    sq = np.ascontiguousarray(_signed_stack(sinT)[:, q0:q0 + SQ])
    ckf = np.vstack([cosT, cosT])
    skf = _signed_stack(sinT)
    if perm is not None:
        ckf = ckf[:, perm]
        skf = skf[:, perm]
    mk = mask.astype(np.float32)
    if perm is not None:
        mk = mk[perm]
    mk = np.concatenate([mk, np.zeros(19 * 128 - SEQ, np.float32)])
    mk = np.ascontiguousarray(mk.reshape(19, 128).T)
    import ml_dtypes
    bf = ml_dtypes.bfloat16
    return {
        "xt": np.ascontiguousarray(
            np.stack([xt[:, i * 580:(i + 1) * 580] for i in range(4)])
        ).astype(bf),
        "wq": wq.astype(bf), "wk": wk.astype(bf),
        "wv": wv.astype(bf), "wp": wp.astype(bf),
        "bq": bq, "bk": bk,
        "cq": cq.astype(bf), "sq": sq.astype(bf),
        "ck": np.ascontiguousarray(ckf).astype(bf),
        "sk": np.ascontiguousarray(skf).astype(bf),
        "mk": np.ascontiguousarray(mk),
        "ones64": np.ones((1, 64), np.float32),
    }


def _run(x, mask, Wqkv, bqkv, Wproj, bproj, trace=False):
    global _prog
    from concourse.bass_utils import run_bass_kernel_spmd
    if _prog is None:
        _prog = _build()
    x = np.asarray(x, np.float32)
    mask = np.asarray(mask)
    Wqkv = np.asarray(Wqkv, np.float32)
    bqkv = np.asarray(bqkv, np.float32)
    Wproj = np.asarray(Wproj, np.float32)
    bproj = np.asarray(bproj, np.float32)
    cos, sin = _rope_tables()
    in_maps = [
        _core_inputs(x, mask, Wqkv, Wproj, bqkv, cos, sin, core // 2, core % 2)
        for core in range(8)
    ]
    res = run_bass_kernel_spmd(_prog, in_maps, list(range(8)), trace=trace)
    acc = np.zeros((SEQ, E), np.float64)
    for core in range(8):
        s = core % 2
        acc[SQ * s:SQ * (s + 1)] += res.results[core]["pout"].astype(np.float64)
    bias_row = bproj.astype(np.float64) + Wproj.astype(np.float64) @ \
        bqkv[1536:2304].astype(np.float64)
    acc += bias_row
    return acc.astype(np.float32), res


def kernel(x, mask, Wqkv, bqkv, Wproj, bproj):
    out, _ = _run(x, mask, Wqkv, bqkv, Wproj, bproj, trace=False)
    return out


# revision 18
# speedup vs baseline: 1.1465x; 1.1465x over previous
"""Multi-head attention (2D-RoPE, masked softmax) on 8 Trainium2 NeuronCores.

Sharding: 4 head-groups (3 heads each) x 2 query-halves (1160 rows each).
Each core computes full attention for its 3 heads over its 1160 query rows
against all 2320 keys, plus its share of the output projection; the host
sums the 8 partial projections and adds the (folded) biases.

Device-side notes:
  - Attention core runs in bf16 (x, Wqkv, K/Q/V, rope tables, exp weights);
    v2: the output projection runs in bf16 as well (ctxn + Wproj) -- the
    f32r path hit the slow fp32 LDWEIGHTS mode.
  - K/Q are produced directly in transposed [head_dim, seq] layout so
    scores come out as scoresT[m, l] with keys on partitions -> softmax
    needs no partition reduction: exp on ScalarE straight out of PSUM
    (0.125 scale folded in; no max-subtraction needed, |s*scale| < ~3),
    denominator via a ones-column appended to V (free PSUM row), and the
    mask folded into V (zeroed rows + mask-valued ones-column, the
    ones-column written once per head instead of per key-chunk).
  - All attention matmuls are zero-padded to K=128 contraction: half-array
    (K=64) matmuls do not register as "busy" for the PE clock monitor and
    the kernel gets stuck at 1.2 GHz; full-array shapes hold 2.4 GHz.
  - Inner loop is software-pipelined over pairs of key-chunks (one exp op
    covers two chunks across two PSUM banks; ctx matmuls trail by two
    pairs) so ScalarE exp overlaps PE work; the 1/Z chain is deferred into
    the next tile's instruction stream (Z row -> reciprocal_approx ->
    K=1 outer-product matmul broadcast -> one DVE multiply).
  - Key order (m) is contraction-internal, so each core's x/K-tables/mask
    are permuted to put its own query rows first; one xT buffer serves
    both the K/V projections (all columns) and the Q projection (first
    1160 columns), keeping the program identical across cores (SPMD).
    x is passed in piece-major layout so every DMA is one contiguous
    block (descriptor-bound otherwise).
  - V-bias and output bias never touch the device:
    out = softmax(..) @ (Vx + bv) @ Wp.T + bp = dev_out + (Wp @ bv + bp).
"""
import sys
if '/opt/trn_rl_repo' not in sys.path:
    sys.path.insert(0, '/opt/trn_rl_repo')
import numpy as np

SEQ, E, NH, D = 2320, 768, 12, 64
GRID, TASK = 48, 16
SQ = SEQ // 2           # query rows per core
HG = 3                  # heads per core
SCALE = D ** -0.5
EC = 6                  # embed chunks of 128
L_TILES = [(0, 512), (512, 392), (904, 256)]
N_TILES = [(0, 512), (512, 512), (1024, 512), (1536, 512), (2048, 272)]
MC = [(i * 128, min(128, SEQ - i * 128)) for i in range(19)]
PT = [(i * 128, min(128, SQ - i * 128)) for i in range(10)]

# Schraudolph exp2 bit-trick (bf16-bits): i16 = s*EXPA + EXPB; bitcast bf16
# ~= C * exp(s*SCALE); the constant C and the +-3% sawtooth wash out in the
# softmax normalization. Used for head1's odd chunk-pairs only (~1/6 of
# weights, ~+1.3e-3 output error).
EXPA = 128.0 * 1.4426950408889634 * SCALE
EXPB = 16251.0

_prog = None


def _build(stages=3):
    import concourse.mybir as mybir
    import concourse.tile as tile
    from concourse import bacc

    F32, F32R = mybir.dt.float32, mybir.dt.float32r
    BF16 = mybir.dt.bfloat16
    I16 = mybir.dt.int16
    AF = mybir.ActivationFunctionType
    ALU = mybir.AluOpType

    nc = bacc.Bacc('TRN2', target_bir_lowering=False, debug=False, num_devices=8)
    dp = nc.declare_dram_parameter
    xt_d = dp("xt", [4, E, 580], BF16, isOutput=False)
    wq_d = dp("wq", [E, 192], BF16, isOutput=False)
    wk_d = dp("wk", [E, 192], BF16, isOutput=False)
    wv_d = dp("wv", [E, 192], BF16, isOutput=False)
    wp_d = dp("wp", [192, E], BF16, isOutput=False)
    bq_d = dp("bq", [128, 2], F32, isOutput=False)
    bk_d = dp("bk", [128, 2], F32, isOutput=False)
    cq_d = dp("cq", [128, SQ], BF16, isOutput=False)
    sq_d = dp("sq", [128, SQ], BF16, isOutput=False)
    ck_d = dp("ck", [128, SEQ], BF16, isOutput=False)
    sk_d = dp("sk", [128, SEQ], BF16, isOutput=False)
    mk_d = dp("mk", [128, 19], F32, isOutput=False)
    on_d = dp("ones64", [1, 64], F32R, isOutput=False)
    out_d = dp("pout", [SQ, E], F32, isOutput=True)

    with tile.TileContext(nc) as tc:
        with (
            tc.tile_pool(name="long", bufs=1) as lp,
            tc.tile_pool(name="zp", bufs=2) as zp,
        ):
            kt01 = lp.tile([128, SEQ], BF16, tag="kt01")
            qt01 = lp.tile([128, SQ], BF16, tag="qt01")
            kt2 = lp.tile([128, SEQ], BF16, tag="kt2")
            qt2 = lp.tile([128, SQ], BF16, tag="qt2")
            rk01 = lp.tile([128, SEQ], BF16, tag="rk01")
            rq01 = lp.tile([128, SQ], BF16, tag="rq01")
            rk2 = lp.tile([64, SEQ], BF16, tag="rk2")
            rq2 = lp.tile([64, SQ], BF16, tag="rq2")
            v_all = lp.tile([128, 19, HG, 65], BF16, tag="v_all")
            wp01 = lp.tile([128, E], BF16, tag="wp01")
            wp2 = lp.tile([64, E], BF16, tag="wp2")
            ctxn01 = lp.tile([128, SQ], BF16, tag="ctxn01")
            ctxn2 = lp.tile([64, SQ], BF16, tag="ctxn2")
            ones64 = lp.tile([1, 64], F32R, tag="ones64")
            mk_sb = lp.tile([128, 19], F32, tag="mk")
            cq_sb = lp.tile([128, SQ], BF16, tag="cq")
            sq_sb = lp.tile([128, SQ], BF16, tag="sq")
            ck_sb = lp.tile([128, SEQ], BF16, tag="ck")
            sk_sb = lp.tile([128, SEQ], BF16, tag="sk")

            warm = lp.tile([128, 128], BF16, tag="warm")
            nc.gpsimd.memset(warm[:], 0.0)

            # small/ordering-critical DMAs first (weights, biases, tables),
            # split into pieces so consumers unblock early
            nc.sync.dma_start(ones64[:], on_d[:])
            nc.sync.dma_start(mk_sb[:], mk_d[:])
            nc.scalar.dma_start(wp01[:], wp_d[0:128, :])
            nc.scalar.dma_start(wp2[:], wp_d[128:192, :])
            nc.gpsimd.memset(kt2[64:128, :], 0.0)
            nc.gpsimd.memset(qt2[64:128, :], 0.0)

            # ---- phase A + attention: pools are scoped so lt0's attention
            # can emit before the V-tiles (which ride inside it), letting the
            # in-order PE queue reach the first scores ~30us earlier. ----
            with tc.tile_pool(name="p12", bufs=1) as p12:
                wq_sb = p12.tile([128, EC, 192], BF16, tag="wq")
                wk_sb = p12.tile([128, EC, 192], BF16, tag="wk")
                wv_sb = p12.tile([128, EC, 192], BF16, tag="wv")
                for w_sb, w_d in ((wk_sb, wk_d), (wq_sb, wq_d), (wv_sb, wv_d)):
                    nc.scalar.dma_start(
                        w_sb[:, :, :],
                        w_d[:, :].rearrange("(c p) w -> p c w", p=128))
                bq_sb = p12.tile([128, 2], F32, tag="bq")
                nc.sync.dma_start(bq_sb[:], bq_d[:])
                bk_sb = p12.tile([128, 2], F32, tag="bk")
                nc.sync.dma_start(bk_sb[:], bk_d[:])
                xt_all = p12.tile([128, EC, SEQ], BF16, tag="xta")
                for pi in range(4):
                    off = pi * 580
                    eng = nc.sync if pi < 2 else nc.scalar
                    eng.dma_start(
                        xt_all[:, :, off:off + 580],
                        xt_d[pi].rearrange("(c p) w -> p c w", p=128))
                nc.gpsimd.dma_start(ck_sb[:], ck_d[:])
                nc.gpsimd.dma_start(sk_sb[:], sk_d[:])
                nc.gpsimd.dma_start(cq_sb[:], cq_d[:])
                nc.gpsimd.dma_start(sq_sb[:], sq_d[:])
                nc.gpsimd.dma_start(wp01[:], wp_d[0:128, :])
                nc.gpsimd.dma_start(wp2[:], wp_d[128:192, :])

                # mask ones-column of v_all, written once per head
                for h in range(HG):
                    nc.vector.tensor_copy(
                        v_all[0:128, 0:19, h, 64:65],
                        mk_sb[0:128, 0:19].rearrange("p (a b) -> p a b", b=1))

                def v_tile(i):
                    off, m = MC[i]
                    pv = pvp.tile([128, 192], F32, tag="pv", name="pv")
                    for c in range(EC):
                        nc.tensor.matmul(
                            pv[0:m, :], xt_all[:, c, off:off + m], wv_sb[:, c, :],
                            start=(c == 0), stop=(c == EC - 1))
                    nc.vector.tensor_mul(
                        v_all[0:m, i, :, 0:64],
                        pv[0:m, 0:192].rearrange("p (h d) -> p h d", h=HG),
                        mk_sb[0:m, i:i + 1].to_broadcast([m, HG, 64]))

                def qk_mm(w_sb, b_sb, grp, off, n, rawt):
                    M = 128 if grp == 0 else 64
                    wc = slice(0, 128) if grp == 0 else slice(128, 192)
                    ps = pkp.tile([128, 512], F32, tag="pk", name="pk")
                    for c in range(EC):
                        nc.tensor.matmul(
                            ps[0:M, 0:n], w_sb[:, c, wc],
                            xt_all[:, c, off:off + n],
                            start=(c == 0), stop=(c == EC - 1))
                    bcol = 0 if grp == 0 else 1
                    nc.scalar.activation(
                        rawt[0:M, off:off + n], ps[0:M, 0:n],
                        AF.Identity,
                        bias=b_sb[0:M, bcol:bcol + 1],
                        scale=1.0)

                def dummy_mm(k=1):
                    for _ in range(k):
                        pwm = pkp.tile([128, 512], F32, tag="pk", name="pwm")
                        nc.tensor.matmul(pwm[0:128, 0:512],
                                         warm[0:128, 0:128],
                                         warm[0:128, 0:512],
                                         start=True, stop=True)

                # ---- attention pools (6 banks) span the K/Q phase and all
                # L-tiles; pkp (2 banks) closes after the K/Q jobs, pvp (2)
                # lives only for lt0's interleaved V-tiles, pp3 (2) after.
                with tc.tile_pool(name="ep", bufs=3) as ep, \
                     tc.tile_pool(name="op", bufs=2) as op, \
                     tc.tile_pool(name="rzp", bufs=4) as rzp, \
                     tc.tile_pool(name="ps3", bufs=2, space="PSUM") as ps3, \
                     tc.tile_pool(name="pc3", bufs=2, space="PSUM") as pc3:
                    with tc.tile_pool(name="pk", bufs=2, space="PSUM") as pkp:
                        # PE warmup on the pkp ring bridges the input-DMA
                        # window at full clock.
                        dummy_mm(16)
                        def piece_of(off, n):
                            return (off + n - 1) // 580
                        mixed = []
                        for pc in range(4):
                            for (off, n) in N_TILES:
                                if piece_of(off, n) == pc:
                                    mixed.append(("k", 0, off, n))
                                    mixed.append(("k", 1, off, n))
                            for (off, n) in L_TILES:
                                if piece_of(off, n) == pc:
                                    mixed.append(("q", 0, off, n))
                                    mixed.append(("q", 1, off, n))
                        for job in mixed:
                            kind, grp, off, n = job
                            if kind == "k":
                                qk_mm(wk_sb, bk_sb, grp, off, n,
                                      rk01 if grp == 0 else rk2)
                                if grp == 1:
                                    rope(rk01, rk01r, ck_sb, sk_sb, 128, off, n,
                                         [kt0, kt1])
                                    rope(rk2, rk2r, ck_sb, sk_sb, 64, off, n,
                                         [kt2])
                            else:
                                qk_mm(wq_sb, bq_sb, grp, off, n,
                                      rq01 if grp == 0 else rq2)
                                if grp == 1:
                                    rope(rq01, rq01r, cq_sb, sq_sb, 128, off, n,
                                         [qt0, qt1])
                                    rope(rq2, rq2r, cq_sb, sq_sb, 64, off, n,
                                         [qt2])

                    PROJ_OF_LT = {0: PT[0:4], 1: PT[4:7], 2: PT[7:10]}
                    PAIRS = [(i, i + 1) if i + 1 < len(MC) else (i,)
                             for i in range(0, len(MC), 2)]
                    pending = []
                    pp3 = None

                    def proj_slice(toff, tm):
                        outsb = op.tile([128, E], F32, tag="outsb", name="outsb")
                        for half in range(2):
                            hs = half * 384
                            pp = pp3.tile([128, 512], F32, tag="pp", name="pp")
                            nc.tensor.matmul(
                                pp[0:tm, 0:384], ctxn01[0:128, toff:toff + tm],
                                wp01[0:128, hs:hs + 384], start=True, stop=False)
                            nc.tensor.matmul(
                                pp[0:tm, 0:384], ctxn2[0:64, toff:toff + tm],
                                wp2[0:64, hs:hs + 384], start=False, stop=True)
                            nc.vector.tensor_copy(outsb[0:tm, hs:hs + 384],
                                                  pp[0:tm, 0:384])
                        nc.sync.dma_start(out_d[toff:toff + tm, :], outsb[0:tm, :])

                    def finish_tile(z):
                        zrow, ctxu, ctxap2, dr0, loff2, ln2 = z
                        zscr = zp.tile([1, 512], F32, tag="zscr", name="zscr")
                        rzf = zp.tile([1, 512], F32, tag="rzf", name="rzf")
                        nc.vector.reciprocal_approx_accurate(
                            rzf[0:1, 0:ln2], zrow[0:1, 0:ln2], zscr[0:1, 0:ln2])
                        rzr = zp.tile([1, 512], F32R, tag="rzr", name="rzr")
                        nc.vector.tensor_copy(rzr[0:1, 0:ln2], rzf[0:1, 0:ln2])
                        przb = pp3.tile([128, 512], F32, tag="pp", name="przb")
                        nc.tensor.matmul(
                            przb[0:64, 0:ln2], ones64[:], rzr[0:1, 0:ln2],
                            start=True, stop=True)
                        rzb = rzp.tile([64, 512], F32, tag="rzb", name="rzb")
                        nc.vector.tensor_copy(rzb[:, 0:ln2], przb[0:64, 0:ln2])
                        nc.vector.tensor_mul(
                            ctxap2[dr0:dr0 + 64, loff2:loff2 + ln2],
                            ctxu[0:64, 0:ln2], rzb[0:64, 0:ln2])

                    def park(pctx, dstt, dr0, loff, ln):
                        zrow = zp.tile([1, 512], F32, tag="zrow", name="zrow",
                                       bufs=4)
                        nc.vector.tensor_copy(zrow[0:1, 0:ln], pctx[64:65, 0:ln])
                        ctxu = rzp.tile([64, 512], F32, tag="ctxu", name="ctxu")
                        nc.vector.tensor_copy(ctxu[:, 0:ln], pctx[0:64, 0:ln])
                        pending.append((zrow, ctxu, dstt, dr0, loff, ln))

                    def exp_tile(ps, ex, m0, ln, two, on_dve=False):
                        if two:
                            dst = ex[0:m0, 0:2 * ln].rearrange(
                                "p (two n) -> p two n", two=2)
                            srcv = ps[0:m0, :].rearrange(
                                "p (two n) -> p two n", two=2)[:, :, 0:ln]
                        else:
                            dst = ex[0:m0, 0:ln]
                            srcv = ps[0:m0, 0:ln]
                        if on_dve:
                            nc.vector.tensor_scalar(
                                dst.bitcast(I16), srcv, EXPA, EXPB,
                                ALU.mult, ALU.add)
                        else:
                            nc.scalar.activation(dst, srcv, AF.Exp,
                                                 bias=0.0, scale=SCALE)

                    def lt_body(lt_i, loff, ln, emit_v):
                        # --- heads 0+1 ---
                        pctx01 = [pc3.tile([65, 512], F32, tag="pctx",
                                           name=f"pctx{h}") for h in range(2)]
                        exs = {}

                        def scores_exp01(p):
                            chunks = PAIRS[p]
                            for h, (ktap, qtap) in enumerate(
                                    ((kt0, qt0), (kt1, qt1))):
                                ps = ps3.tile([128, 1024], F32, tag="ps",
                                              name=f"ps{h}")
                                for j, i in enumerate(chunks):
                                    moff, m = MC[i]
                                    kk = 128 if m == 128 else 64
                                    nc.tensor.matmul(
                                        ps[0:m, j * 512:j * 512 + ln],
                                        ktap[0:kk, moff:moff + m],
                                        qtap[0:kk, loff:loff + ln],
                                        start=True, stop=True)
                                ex = ep.tile([128, 1024], BF16, tag="ex",
                                             name=f"ex{h}")
                                exp_tile(ps, ex, MC[chunks[0]][1], ln,
                                         len(chunks) == 2,
                                         on_dve=(h == 1 and p % 2 == 1))
                                exs[(p, h)] = ex

                        def ctx01(p):
                            for h in range(2):
                                ex = exs.pop((p, h))
                                for j, i in enumerate(PAIRS[p]):
                                    moff, m = MC[i]
                                    nc.tensor.matmul(
                                        pctx01[h][:, 0:ln], v_all[0:m, i, h, :],
                                        ex[0:m, j * ln:j * ln + ln],
                                        start=(i == 0), stop=(i == len(MC) - 1))

                        if emit_v:
                            for i in range(4):
                                v_tile(i)
                        for p in range(len(PAIRS) + 2):
                            if emit_v:
                                for i in (2 * p + 4, 2 * p + 5):
                                    if i < len(MC):
                                        v_tile(i)
                            if p < len(PAIRS):
                                scores_exp01(p)
                            if p >= 2:
                                ctx01(p - 2)
                        park(pctx01[0], ctxn01, 0, loff, ln)
                        park(pctx01[1], ctxn01, 64, loff, ln)

                        # --- head 2: zero-padded K=128 path ---
                        pctx2 = pc3.tile([65, 512], F32, tag="pctx",
                                         name="pctx2")
                        exs2 = {}

                        def scores_exp2(p):
                            chunks = PAIRS[p]
                            ps = ps3.tile([128, 1024], F32, tag="ps", name="ps2")
                            for j, i in enumerate(chunks):
                                moff, m = MC[i]
                                kk = 128 if m == 128 else 64
                                nc.tensor.matmul(
                                    ps[0:m, j * 512:j * 512 + ln],
                                    kt2[0:kk, moff:moff + m],
                                    qt2[0:kk, loff:loff + ln],
                                    start=True, stop=True)
                            ex = ep.tile([128, 1024], BF16, tag="ex", name="ex2")
                            exp_tile(ps, ex, MC[chunks[0]][1], ln,
                                     len(chunks) == 2)
                            exs2[p] = ex

                        def ctx2(p):
                            ex = exs2.pop(p)
                            for j, i in enumerate(PAIRS[p]):
                                moff, m = MC[i]
                                nc.tensor.matmul(
                                    pctx2[:, 0:ln], v_all[0:m, i, 2, :],
                                    ex[0:m, j * ln:j * ln + ln],
                                    start=(i == 0), stop=(i == len(MC) - 1))

                        for p in range(len(PAIRS) + 2):
                            if p < len(PAIRS):
                                scores_exp2(p)
                            if p >= 2:
                                ctx2(p - 2)
                        park(pctx2, ctxn2, 0, loff, ln)

                    def flush_and_proj(lt_i):
                        while pending:
                            finish_tile(pending.pop(0))
                        for (toff, tm) in PROJ_OF_LT[lt_i]:
                            proj_slice(toff, tm)

                    with tc.tile_pool(name="pv", bufs=2, space="PSUM") as pvp:
                        lt_body(0, L_TILES[0][0], L_TILES[0][1], True)
                    with tc.tile_pool(name="pp3", bufs=2, space="PSUM") as pp3:
                        flush_and_proj(0)
                        for lt_i in (1, 2):
                            lt_body(lt_i, L_TILES[lt_i][0], L_TILES[lt_i][1],
                                    False)
                            flush_and_proj(lt_i)

    nc.finalize()
    return nc


def _rope_tables():
    dim = D // 2
    freqs = 1.0 / 10000 ** (np.arange(0, dim, 2, dtype=np.float64) / dim)
    t = np.arange(GRID, dtype=np.float64)
    f = np.repeat(np.outer(t, freqs), 2, axis=-1)                  # [48, 32]
    fr = np.broadcast_to(f[:, None, :], (GRID, GRID, dim))
    fc = np.broadcast_to(f[None, :, :], (GRID, GRID, dim))
    full = np.concatenate([fr, fc], axis=-1).reshape(GRID * GRID, D)
    cos = np.ones((SEQ, D), np.float64)
    sin = np.zeros((SEQ, D), np.float64)
    cos[TASK:] = np.cos(full)
    sin[TASK:] = np.sin(full)
    return cos.astype(np.float32), sin.astype(np.float32)


def _signed_stack(tT):
    # [64, S] -> [128, S]: signed sine table stored at the ROTATED (source)
    # rows, so the rope half-multiplies read both operands at equal partition
    # bases: sinB[32:64] = -sin[0:32], sinB[0:32] = +sin[32:64], stacked x2.
    s = np.vstack([tT[32:64], -tT[0:32]])
    return np.ascontiguousarray(np.vstack([s, s]))


def _core_inputs(x, mask, Wqkv, Wproj, bqkv, cos, sin, g, s):
    xT = x.T  # [768, 2320]
    q0 = SQ * s
    if s == 0:
        perm = None
        xt = np.ascontiguousarray(xT)
    else:
        perm = np.concatenate([np.arange(SQ, SEQ), np.arange(0, SQ)])
        xt = np.ascontiguousarray(np.concatenate([xT[:, SQ:], xT[:, :SQ]], axis=1))
    r0 = 192 * g
    wq = np.ascontiguousarray(Wqkv[r0:r0 + 192, :].T)
    wk = np.ascontiguousarray(Wqkv[768 + r0:768 + r0 + 192, :].T)
    wv = np.ascontiguousarray(Wqkv[1536 + r0:1536 + r0 + 192, :].T)
    wp = np.ascontiguousarray(Wproj[:, r0:r0 + 192].T)
    bq = np.zeros((128, 2), np.float32)
    bq[:, 0] = bqkv[r0:r0 + 128]
    bq[0:64, 1] = bqkv[r0 + 128:r0 + 192]
    bk = np.zeros((128, 2), np.float32)
    bk[:, 0] = bqkv[768 + r0:768 + r0 + 128]
    bk[0:64, 1] = bqkv[768 + r0 + 128:768 + r0 + 192]
    cosT, sinT = cos.T, sin.T  # [64, S]
    cq = np.ascontiguousarray(np.vstack([cosT, cosT])[:, q0:q0 + SQ])
    sq = np.ascontiguousarray(_signed_stack(sinT)[:, q0:q0 + SQ])
    ckf = np.vstack([cosT, cosT])
    skf = _signed_stack(sinT)
    if perm is not None:
        ckf = ckf[:, perm]
        skf = skf[:, perm]
    mk = mask.astype(np.float32)
    if perm is not None:
        mk = mk[perm]
    mk = np.concatenate([mk, np.zeros(19 * 128 - SEQ, np.float32)])
    mk = np.ascontiguousarray(mk.reshape(19, 128).T)
    import ml_dtypes
    bf = ml_dtypes.bfloat16
    return {
        "xt": np.ascontiguousarray(
            np.stack([xt[:, i * 580:(i + 1) * 580] for i in range(4)])
        ).astype(bf),
        "wq": wq.astype(bf), "wk": wk.astype(bf),
        "wv": wv.astype(bf), "wp": wp.astype(bf),
        "bq": bq, "bk": bk,
        "cq": cq.astype(bf), "sq": sq.astype(bf),
        "ck": np.ascontiguousarray(ckf).astype(bf),
        "sk": np.ascontiguousarray(skf).astype(bf),
        "mk": np.ascontiguousarray(mk),
        "ones64": np.ones((1, 64), np.float32),
    }


def _run(x, mask, Wqkv, bqkv, Wproj, bproj, trace=False):
    global _prog
    from concourse.bass_utils import run_bass_kernel_spmd
    if _prog is None:
        _prog = _build()
    x = np.asarray(x, np.float32)
    mask = np.asarray(mask)
    Wqkv = np.asarray(Wqkv, np.float32)
    bqkv = np.asarray(bqkv, np.float32)
    Wproj = np.asarray(Wproj, np.float32)
    bproj = np.asarray(bproj, np.float32)
    cos, sin = _rope_tables()
    in_maps = [
        _core_inputs(x, mask, Wqkv, Wproj, bqkv, cos, sin, core // 2, core % 2)
        for core in range(8)
    ]
    res = run_bass_kernel_spmd(_prog, in_maps, list(range(8)), trace=trace)
    acc = np.zeros((SEQ, E), np.float64)
    for core in range(8):
        s = core % 2
        acc[SQ * s:SQ * (s + 1)] += res.results[core]["pout"].astype(np.float64)
    bias_row = bproj.astype(np.float64) + Wproj.astype(np.float64) @ \
        bqkv[1536:2304].astype(np.float64)
    acc += bias_row
    return acc.astype(np.float32), res


def kernel(x, mask, Wqkv, bqkv, Wproj, bproj):
    out, _ = _run(x, mask, Wqkv, bqkv, Wproj, bproj, trace=False)
    return out


# revision 20
# speedup vs baseline: 1.1581x; 1.0101x over previous
"""Multi-head attention (2D-RoPE, masked softmax) on 8 Trainium2 NeuronCores.

Sharding: 4 head-groups (3 heads each) x 2 query-halves (1160 rows each).
Each core computes full attention for its 3 heads over its 1160 query rows
against all 2320 keys, plus its share of the output projection; the host
sums the 8 partial projections and adds the (folded) biases.

Device-side notes:
  - Attention core runs in bf16 (x, Wqkv, K/Q/V, rope tables, exp weights);
    v2: the output projection runs in bf16 as well (ctxn + Wproj) -- the
    f32r path hit the slow fp32 LDWEIGHTS mode.
  - K/Q are produced directly in transposed [head_dim, seq] layout so
    scores come out as scoresT[m, l] with keys on partitions -> softmax
    needs no partition reduction: exp on ScalarE straight out of PSUM
    (0.125 scale folded in; no max-subtraction needed, |s*scale| < ~3),
    denominator via a ones-column appended to V (free PSUM row), and the
    mask folded into V (zeroed rows + mask-valued ones-column, the
    ones-column written once per head instead of per key-chunk).
  - All attention matmuls are zero-padded to K=128 contraction: half-array
    (K=64) matmuls do not register as "busy" for the PE clock monitor and
    the kernel gets stuck at 1.2 GHz; full-array shapes hold 2.4 GHz.
  - Inner loop is software-pipelined over pairs of key-chunks (one exp op
    covers two chunks across two PSUM banks; ctx matmuls trail by two
    pairs) so ScalarE exp overlaps PE work; the 1/Z chain is deferred into
    the next tile's instruction stream (Z row -> reciprocal_approx ->
    K=1 outer-product matmul broadcast -> one DVE multiply).
  - Key order (m) is contraction-internal, so each core's x/K-tables/mask
    are permuted to put its own query rows first; one xT buffer serves
    both the K/V projections (all columns) and the Q projection (first
    1160 columns), keeping the program identical across cores (SPMD).
    x is passed in piece-major layout so every DMA is one contiguous
    block (descriptor-bound otherwise).
  - V-bias and output bias never touch the device:
    out = softmax(..) @ (Vx + bv) @ Wp.T + bp = dev_out + (Wp @ bv + bp).
"""
import sys
if '/opt/trn_rl_repo' not in sys.path:
    sys.path.insert(0, '/opt/trn_rl_repo')
import numpy as np

SEQ, E, NH, D = 2320, 768, 12, 64
GRID, TASK = 48, 16
SQ = SEQ // 2           # query rows per core
HG = 3                  # heads per core
SCALE = D ** -0.5
EC = 6                  # embed chunks of 128
L_TILES = [(0, 512), (512, 392), (904, 256)]
N_TILES = [(0, 512), (512, 512), (1024, 512), (1536, 512), (2048, 272)]
MC = [(i * 128, min(128, SEQ - i * 128)) for i in range(19)]
PT = [(i * 128, min(128, SQ - i * 128)) for i in range(10)]

_prog = None


def _build(stages=3):
    import concourse.mybir as mybir
    import concourse.tile as tile
    from concourse import bacc

    F32, F32R = mybir.dt.float32, mybir.dt.float32r
    BF16 = mybir.dt.bfloat16
    AF = mybir.ActivationFunctionType

    nc = bacc.Bacc('TRN2', target_bir_lowering=False, debug=False, num_devices=8)
    dp = nc.declare_dram_parameter
    xt_d = dp("xt", [4, E, 580], BF16, isOutput=False)
    wq_d = dp("wq", [E, 192], BF16, isOutput=False)
    wk_d = dp("wk", [E, 192], BF16, isOutput=False)
    wv_d = dp("wv", [E, 192], BF16, isOutput=False)
    wp_d = dp("wp", [192, E], BF16, isOutput=False)
    bq_d = dp("bq", [128, 2], F32, isOutput=False)
    bk_d = dp("bk", [128, 2], F32, isOutput=False)
    cq_d = dp("cq", [128, SQ], BF16, isOutput=False)
    sq_d = dp("sq", [128, SQ], BF16, isOutput=False)
    ck_d = dp("ck", [128, SEQ], BF16, isOutput=False)
    sk_d = dp("sk", [128, SEQ], BF16, isOutput=False)
    mk_d = dp("mk", [128, 19], F32, isOutput=False)
    on_d = dp("ones64", [1, 64], F32R, isOutput=False)
    out_d = dp("pout", [SQ, E], F32, isOutput=True)

    with tile.TileContext(nc) as tc:
        with (
            tc.tile_pool(name="long", bufs=1) as lp,
            tc.tile_pool(name="zp", bufs=2) as zp,
        ):
            kt01 = lp.tile([128, SEQ], BF16, tag="kt01")
            qt01 = lp.tile([128, SQ], BF16, tag="qt01")
            kt2 = lp.tile([128, SEQ], BF16, tag="kt2")
            qt2 = lp.tile([128, SQ], BF16, tag="qt2")
            rk01 = lp.tile([128, SEQ], BF16, tag="rk01")
            rq01 = lp.tile([128, SQ], BF16, tag="rq01")
            rk2 = lp.tile([64, SEQ], BF16, tag="rk2")
            rq2 = lp.tile([64, SQ], BF16, tag="rq2")
            v_all = lp.tile([128, 19, HG, 65], BF16, tag="v_all")
            wp01 = lp.tile([128, E], BF16, tag="wp01")
            wp2 = lp.tile([64, E], BF16, tag="wp2")
            ctxn01 = lp.tile([128, SQ], BF16, tag="ctxn01")
            ctxn2 = lp.tile([64, SQ], BF16, tag="ctxn2")
            ones64 = lp.tile([1, 64], F32R, tag="ones64")
            mk_sb = lp.tile([128, 19], F32, tag="mk")
            cq_sb = lp.tile([128, SQ], BF16, tag="cq")
            sq_sb = lp.tile([128, SQ], BF16, tag="sq")
            ck_sb = lp.tile([128, SEQ], BF16, tag="ck")
            sk_sb = lp.tile([128, SEQ], BF16, tag="sk")

            warm = lp.tile([128, 128], BF16, tag="warm")
            nc.gpsimd.memset(warm[:], 0.0)

            # small/ordering-critical DMAs first (weights, biases, tables),
            # split into pieces so consumers unblock early
            nc.sync.dma_start(ones64[:], on_d[:])
            nc.sync.dma_start(mk_sb[:], mk_d[:])
            nc.scalar.dma_start(wp01[:], wp_d[0:128, :])
            nc.scalar.dma_start(wp2[:], wp_d[128:192, :])
            nc.gpsimd.memset(kt2[64:128, :], 0.0)
            nc.gpsimd.memset(qt2[64:128, :], 0.0)

            # ---- phase A + attention: pools are scoped so lt0's attention
            # can emit before the V-tiles (which ride inside it), letting the
            # in-order PE queue reach the first scores ~30us earlier. ----
            with tc.tile_pool(name="p12", bufs=1) as p12:
                wq_sb = p12.tile([128, EC, 192], BF16, tag="wq")
                wk_sb = p12.tile([128, EC, 192], BF16, tag="wk")
                wv_sb = p12.tile([128, EC, 192], BF16, tag="wv")
                for w_sb, w_d in ((wk_sb, wk_d), (wq_sb, wq_d), (wv_sb, wv_d)):
                    nc.scalar.dma_start(
                        w_sb[:, :, :],
                        w_d[:, :].rearrange("(c p) w -> p c w", p=128))
                bq_sb = p12.tile([128, 2], F32, tag="bq")
                nc.sync.dma_start(bq_sb[:], bq_d[:])
                bk_sb = p12.tile([128, 2], F32, tag="bk")
                nc.sync.dma_start(bk_sb[:], bk_d[:])
                xt_all = p12.tile([128, EC, SEQ], BF16, tag="xta")
                for pi in range(4):
                    off = pi * 580
                    eng = nc.sync if pi < 2 else nc.scalar
                    eng.dma_start(
                        xt_all[:, :, off:off + 580],
                        xt_d[pi].rearrange("(c p) w -> p c w", p=128))
                nc.gpsimd.dma_start(ck_sb[:], ck_d[:])
                nc.gpsimd.dma_start(sk_sb[:], sk_d[:])
                nc.gpsimd.dma_start(cq_sb[:], cq_d[:])
                nc.gpsimd.dma_start(sq_sb[:], sq_d[:])
                nc.gpsimd.dma_start(wp01[:], wp_d[0:128, :])
                nc.gpsimd.dma_start(wp2[:], wp_d[128:192, :])

                # mask ones-column of v_all, written once per head
                for h in range(HG):
                    nc.vector.tensor_copy(
                        v_all[0:128, 0:19, h, 64:65],
                        mk_sb[0:128, 0:19].rearrange("p (a b) -> p a b", b=1))

                def v_tile(i):
                    off, m = MC[i]
                    pv = pvp.tile([128, 192], F32, tag="pv", name="pv")
                    for c in range(EC):
                        nc.tensor.matmul(
                            pv[0:m, :], xt_all[:, c, off:off + m], wv_sb[:, c, :],
                            start=(c == 0), stop=(c == EC - 1))
                    nc.vector.tensor_mul(
                        v_all[0:m, i, :, 0:64],
                        pv[0:m, 0:192].rearrange("p (h d) -> p h d", h=HG),
                        mk_sb[0:m, i:i + 1].to_broadcast([m, HG, 64]))

                def qk_mm(w_sb, b_sb, grp, off, n, rawt):
                    M = 128 if grp == 0 else 64
                    wc = slice(0, 128) if grp == 0 else slice(128, 192)
                    ps = pkp.tile([128, 512], F32, tag="pk", name="pk")
                    for c in range(EC):
                        nc.tensor.matmul(
                            ps[0:M, 0:n], w_sb[:, c, wc],
                            xt_all[:, c, off:off + n],
                            start=(c == 0), stop=(c == EC - 1))
                    bcol = 0 if grp == 0 else 1
                    nc.scalar.activation(
                        rawt[0:M, off:off + n], ps[0:M, 0:n],
                        AF.Identity,
                        bias=b_sb[0:M, bcol:bcol + 1],
                        scale=1.0)

                def dummy_mm(k=1):
                    for _ in range(k):
                        pwm = pkp.tile([128, 512], F32, tag="pk", name="pwm")
                        nc.tensor.matmul(pwm[0:128, 0:512],
                                         warm[0:128, 0:128],
                                         warm[0:128, 0:512],
                                         start=True, stop=True)

                # ---- attention pools (6 banks) span the K/Q phase and all
                # L-tiles; pkp (2 banks) closes after the K/Q jobs, pvp (2)
                # lives only for lt0's interleaved V-tiles, pp3 (2) after.
                with tc.tile_pool(name="ep", bufs=3) as ep, \
                     tc.tile_pool(name="op", bufs=2) as op, \
                     tc.tile_pool(name="rzp", bufs=4) as rzp, \
                     tc.tile_pool(name="ps3", bufs=2, space="PSUM") as ps3, \
                     tc.tile_pool(name="pc3", bufs=2, space="PSUM") as pc3:
                    with tc.tile_pool(name="pk", bufs=2, space="PSUM") as pkp:
                        # PE warmup on the pkp ring bridges the input-DMA
                        # window at full clock.
                        dummy_mm(16)
                        def piece_of(off, n):
                            return (off + n - 1) // 580
                        mixed = []
                        for pc in range(4):
                            for (off, n) in N_TILES:
                                if piece_of(off, n) == pc:
                                    mixed.append(("k", 0, off, n))
                                    mixed.append(("k", 1, off, n))
                            for (off, n) in L_TILES:
                                if piece_of(off, n) == pc:
                                    mixed.append(("q", 0, off, n))
                                    mixed.append(("q", 1, off, n))
                        for job in mixed:
                            kind, grp, off, n = job
                            if kind == "k":
                                qk_mm(wk_sb, bk_sb, grp, off, n,
                                      rk01 if grp == 0 else rk2)
                                if grp == 1:
                                    rope(rk01, rk01r, ck_sb, sk_sb, 128, off, n,
                                         [kt0, kt1])
                                    rope(rk2, rk2r, ck_sb, sk_sb, 64, off, n,
                                         [kt2])
                            else:
                                qk_mm(wq_sb, bq_sb, grp, off, n,
                                      rq01 if grp == 0 else rq2)
                                if grp == 1:
                                    rope(rq01, rq01r, cq_sb, sq_sb, 128, off, n,
                                         [qt0, qt1])
                                    rope(rq2, rq2r, cq_sb, sq_sb, 64, off, n,
                                         [qt2])

                    PROJ_OF_LT = {0: PT[0:4], 1: PT[4:7], 2: PT[7:10]}
                    PAIRS = [(i, i + 1) if i + 1 < len(MC) else (i,)
                             for i in range(0, len(MC), 2)]
                    pending = []
                    pp3 = None

                    def proj_slice(toff, tm):
                        outsb = op.tile([128, E], F32, tag="outsb", name="outsb")
                        for half in range(2):
                            hs = half * 384
                            pp = pp3.tile([128, 512], F32, tag="pp", name="pp")
                            nc.tensor.matmul(
                                pp[0:tm, 0:384], ctxn01[0:128, toff:toff + tm],
                                wp01[0:128, hs:hs + 384], start=True, stop=False)
                            nc.tensor.matmul(
                                pp[0:tm, 0:384], ctxn2[0:64, toff:toff + tm],
                                wp2[0:64, hs:hs + 384], start=False, stop=True)
                            nc.vector.tensor_copy(outsb[0:tm, hs:hs + 384],
                                                  pp[0:tm, 0:384])
                        nc.sync.dma_start(out_d[toff:toff + tm, :], outsb[0:tm, :])

                    def finish_tile(z):
                        zrow, ctxu, ctxap2, dr0, loff2, ln2 = z
                        zscr = zp.tile([1, 512], F32, tag="zscr", name="zscr")
                        rzf = zp.tile([1, 512], F32, tag="rzf", name="rzf")
                        nc.vector.reciprocal_approx_accurate(
                            rzf[0:1, 0:ln2], zrow[0:1, 0:ln2], zscr[0:1, 0:ln2])
                        rzr = zp.tile([1, 512], F32R, tag="rzr", name="rzr")
                        nc.vector.tensor_copy(rzr[0:1, 0:ln2], rzf[0:1, 0:ln2])
                        przb = pp3.tile([128, 512], F32, tag="pp", name="przb")
                        nc.tensor.matmul(
                            przb[0:64, 0:ln2], ones64[:], rzr[0:1, 0:ln2],
                            start=True, stop=True)
                        rzb = rzp.tile([64, 512], F32, tag="rzb", name="rzb")
                        nc.vector.tensor_copy(rzb[:, 0:ln2], przb[0:64, 0:ln2])
                        nc.vector.tensor_mul(
                            ctxap2[dr0:dr0 + 64, loff2:loff2 + ln2],
                            ctxu[0:64, 0:ln2], rzb[0:64, 0:ln2])

                    def park(pctx, dstt, dr0, loff, ln):
                        zrow = zp.tile([1, 512], F32, tag="zrow", name="zrow",
                                       bufs=4)
                        nc.vector.tensor_copy(zrow[0:1, 0:ln], pctx[64:65, 0:ln])
                        ctxu = rzp.tile([64, 512], F32, tag="ctxu", name="ctxu")
                        nc.vector.tensor_copy(ctxu[:, 0:ln], pctx[0:64, 0:ln])
                        pending.append((zrow, ctxu, dstt, dr0, loff, ln))

                    def exp_tile(ps, ex, m0, ln, two):
                        if two:
                            nc.scalar.activation(
                                ex[0:m0, 0:2 * ln].rearrange(
                                    "p (two n) -> p two n", two=2),
                                ps[0:m0, :].rearrange(
                                    "p (two n) -> p two n", two=2)[:, :, 0:ln],
                                AF.Exp, bias=0.0, scale=SCALE)
                        else:
                            nc.scalar.activation(
                                ex[0:m0, 0:ln], ps[0:m0, 0:ln], AF.Exp,
                                bias=0.0, scale=SCALE)

                    def lt_h01(lt_i, emit_v=False):
                        loff, ln = L_TILES[lt_i]
                        # --- heads 0+1 ---
                        pctx01 = [pc3.tile([65, 512], F32, tag="pctx",
                                           name=f"pctx{h}") for h in range(2)]
                        exs = {}

                        def scores_exp01(p):
                            chunks = PAIRS[p]
                            for h, (ktap, qtap) in enumerate(
                                    ((kt0, qt0), (kt1, qt1))):
                                ps = ps3.tile([128, 1024], F32, tag="ps",
                                              name=f"ps{h}")
                                for j, i in enumerate(chunks):
                                    moff, m = MC[i]
                                    kk = 128 if m == 128 else 64
                                    nc.tensor.matmul(
                                        ps[0:m, j * 512:j * 512 + ln],
                                        ktap[0:kk, moff:moff + m],
                                        qtap[0:kk, loff:loff + ln],
                                        start=True, stop=True)
                                ex = ep.tile([128, 1024], BF16, tag="ex",
                                             name=f"ex{h}")
                                exp_tile(ps, ex, MC[chunks[0]][1], ln,
                                         len(chunks) == 2)
                                exs[(p, h)] = ex

                        def ctx01(p):
                            for h in range(2):
                                ex = exs.pop((p, h))
                                for j, i in enumerate(PAIRS[p]):
                                    moff, m = MC[i]
                                    nc.tensor.matmul(
                                        pctx01[h][:, 0:ln], v_all[0:m, i, h, :],
                                        ex[0:m, j * ln:j * ln + ln],
                                        start=(i == 0), stop=(i == len(MC) - 1))

                        if emit_v:
                            for i in range(4):
                                v_tile(i)
                        for p in range(len(PAIRS) + 2):
                            if emit_v:
                                for i in (2 * p + 4, 2 * p + 5):
                                    if i < len(MC):
                                        v_tile(i)
                            if p < len(PAIRS):
                                scores_exp01(p)
                            if p >= 2:
                                ctx01(p - 2)
                        park(pctx01[0], ctxn01, 0, loff, ln)
                        park(pctx01[1], ctxn01, 64, loff, ln)

                    def lt_h2(lt_i):
                        loff, ln = L_TILES[lt_i]
                        # --- head 2: zero-padded K=128 path ---
                        pctx2 = pc3.tile([65, 512], F32, tag="pctx",
                                         name="pctx2")
                        exs2 = {}

                        def scores_exp2(p):
                            chunks = PAIRS[p]
                            ps = ps3.tile([128, 1024], F32, tag="ps", name="ps2")
                            for j, i in enumerate(chunks):
                                moff, m = MC[i]
                                kk = 128 if m == 128 else 64
                                nc.tensor.matmul(
                                    ps[0:m, j * 512:j * 512 + ln],
                                    kt2[0:kk, moff:moff + m],
                                    qt2[0:kk, loff:loff + ln],
                                    start=True, stop=True)
                            ex = ep.tile([128, 1024], BF16, tag="ex", name="ex2")
                            exp_tile(ps, ex, MC[chunks[0]][1], ln,
                                     len(chunks) == 2)
                            exs2[p] = ex

                        def ctx2(p):
                            ex = exs2.pop(p)
                            for j, i in enumerate(PAIRS[p]):
                                moff, m = MC[i]
                                nc.tensor.matmul(
                                    pctx2[:, 0:ln], v_all[0:m, i, 2, :],
                                    ex[0:m, j * ln:j * ln + ln],
                                    start=(i == 0), stop=(i == len(MC) - 1))

                        for p in range(len(PAIRS) + 2):
                            if p < len(PAIRS):
                                scores_exp2(p)
                            if p >= 2:
                                ctx2(p - 2)
                        park(pctx2, ctxn2, 0, loff, ln)

                    def flush_and_proj(lt_i):
                        while pending:
                            finish_tile(pending.pop(0))
                        for (toff, tm) in PROJ_OF_LT[lt_i]:
                            proj_slice(toff, tm)

                    with tc.tile_pool(name="pv", bufs=2, space="PSUM") as pvp:
                        lt_h01(0, True)
                        lt_h2(0)
                    with tc.tile_pool(name="pp3", bufs=2, space="PSUM") as pp3:
                        # each flush+proj is emitted AFTER the next tile's
                        # head-pair section so its PE work fills the
                        # exp-paced slack instead of gapping the stream.
                        lt_h01(1)
                        flush_and_proj(0)
                        lt_h2(1)
                        lt_h01(2)
                        flush_and_proj(1)
                        lt_h2(2)
                        flush_and_proj(2)

    nc.finalize()
    return nc


def _rope_tables():
    dim = D // 2
    freqs = 1.0 / 10000 ** (np.arange(0, dim, 2, dtype=np.float64) / dim)
    t = np.arange(GRID, dtype=np.float64)
    f = np.repeat(np.outer(t, freqs), 2, axis=-1)                  # [48, 32]
    fr = np.broadcast_to(f[:, None, :], (GRID, GRID, dim))
    fc = np.broadcast_to(f[None, :, :], (GRID, GRID, dim))
    full = np.concatenate([fr, fc], axis=-1).reshape(GRID * GRID, D)
    cos = np.ones((SEQ, D), np.float64)
    sin = np.zeros((SEQ, D), np.float64)
    cos[TASK:] = np.cos(full)
    sin[TASK:] = np.sin(full)
    return cos.astype(np.float32), sin.astype(np.float32)


def _signed_stack(tT):
    # [64, S] -> [128, S]: signed sine table stored at the ROTATED (source)
    # rows, so the rope half-multiplies read both operands at equal partition
    # bases: sinB[32:64] = -sin[0:32], sinB[0:32] = +sin[32:64], stacked x2.
    s = np.vstack([tT[32:64], -tT[0:32]])
    return np.ascontiguousarray(np.vstack([s, s]))


def _core_inputs(x, mask, Wqkv, Wproj, bqkv, cos, sin, g, s):
    xT = x.T  # [768, 2320]
    q0 = SQ * s
    if s == 0:
        perm = None
        xt = np.ascontiguousarray(xT)
    else:
        perm = np.concatenate([np.arange(SQ, SEQ), np.arange(0, SQ)])
        xt = np.ascontiguousarray(np.concatenate([xT[:, SQ:], xT[:, :SQ]], axis=1))
    r0 = 192 * g
    wq = np.ascontiguousarray(Wqkv[r0:r0 + 192, :].T)
    wk = np.ascontiguousarray(Wqkv[768 + r0:768 + r0 + 192, :].T)
    wv = np.ascontiguousarray(Wqkv[1536 + r0:1536 + r0 + 192, :].T)
    wp = np.ascontiguousarray(Wproj[:, r0:r0 + 192].T)
    bq = np.zeros((128, 2), np.float32)
    bq[:, 0] = bqkv[r0:r0 + 128]
    bq[0:64, 1] = bqkv[r0 + 128:r0 + 192]
    bk = np.zeros((128, 2), np.float32)
    bk[:, 0] = bqkv[768 + r0:768 + r0 + 128]
    bk[0:64, 1] = bqkv[768 + r0 + 128:768 + r0 + 192]
    cosT, sinT = cos.T, sin.T  # [64, S]
    cq = np.ascontiguousarray(np.vstack([cosT, cosT])[:, q0:q0 + SQ])
    sq = np.ascontiguousarray(_signed_stack(sinT)[:, q0:q0 + SQ])
    ckf = np.vstack([cosT, cosT])
    skf = _signed_stack(sinT)
    if perm is not None:
        ckf = ckf[:, perm]
        skf = skf[:, perm]
    mk = mask.astype(np.float32)
    if perm is not None:
        mk = mk[perm]
    mk = np.concatenate([mk, np.zeros(19 * 128 - SEQ, np.float32)])
    mk = np.ascontiguousarray(mk.reshape(19, 128).T)
    import ml_dtypes
    bf = ml_dtypes.bfloat16
    return {
        "xt": np.ascontiguousarray(
            np.stack([xt[:, i * 580:(i + 1) * 580] for i in range(4)])
        ).astype(bf),
        "wq": wq.astype(bf), "wk": wk.astype(bf),
        "wv": wv.astype(bf), "wp": wp.astype(bf),
        "bq": bq, "bk": bk,
        "cq": cq.astype(bf), "sq": sq.astype(bf),
        "ck": np.ascontiguousarray(ckf).astype(bf),
        "sk": np.ascontiguousarray(skf).astype(bf),
        "mk": np.ascontiguousarray(mk),
        "ones64": np.ones((1, 64), np.float32),
    }


def _run(x, mask, Wqkv, bqkv, Wproj, bproj, trace=False):
    global _prog
    from concourse.bass_utils import run_bass_kernel_spmd
    if _prog is None:
        _prog = _build()
    x = np.asarray(x, np.float32)
    mask = np.asarray(mask)
    Wqkv = np.asarray(Wqkv, np.float32)
    bqkv = np.asarray(bqkv, np.float32)
    Wproj = np.asarray(Wproj, np.float32)
    bproj = np.asarray(bproj, np.float32)
    cos, sin = _rope_tables()
    in_maps = [
        _core_inputs(x, mask, Wqkv, Wproj, bqkv, cos, sin, core // 2, core % 2)
        for core in range(8)
    ]
    res = run_bass_kernel_spmd(_prog, in_maps, list(range(8)), trace=trace)
    acc = np.zeros((SEQ, E), np.float64)
    for core in range(8):
        s = core % 2
        acc[SQ * s:SQ * (s + 1)] += res.results[core]["pout"].astype(np.float64)
    bias_row = bproj.astype(np.float64) + Wproj.astype(np.float64) @ \
        bqkv[1536:2304].astype(np.float64)
    acc += bias_row
    return acc.astype(np.float32), res


def kernel(x, mask, Wqkv, bqkv, Wproj, bproj):
    out, _ = _run(x, mask, Wqkv, bqkv, Wproj, bproj, trace=False)
    return out
